# revision 23
# baseline (speedup 1.0000x reference)
"""Trainium2 Bass kernel: per-sample 64-bin histogram + normalize + tiny MLP.

Input  grad_map [128, 512, 512] f32, W1 [32,64], b1 [32], W2 [128,32], b2 [128]
Output [128, 128] f32 = relu(hist_norm @ W1.T + b1) @ W2.T + b2
Sharding: pure data parallel over batch across 8 cores (16 samples/core).

Strategy: 64 bins = 8 hi x 8 lo. Per sample, build 8+8 cumulative step
planes (hi: idx>=8a, lo: (idx&7)>=b, plane 0 = ones); the joint counts
C2[a,b] = #(hi>=a & lo>=b) are then an outer-product reduction computed on
the TensorEngine as 128 accumulating [128x128]@[128x128] bf16 matmuls
(f-interleaved group packing, 16 groups per matmul). The 64-bin histogram
is the 2D finite difference of C2, folded linearly into the MLP tail.
Four hi planes are built on ScalarE as +-1 signs (engine balance); the
resulting affine distortion of C2 rows is corrected for free inside the
host-precomputed left-difference matrix dtd.

Per sample ([128, 2048] f32 tile = one sample's 262144 elements):
  idx = floor(f32(x*64/255)) as int16 (1 VE pass)
  lo  = idx & 7                        (1 VE pass)
  SH[:, a, :] = (idx >= 8a) bf16, a=1..7 ; SH[:, 0, :] = ones   (7 VE passes)
  SL[:, b, :] = (lo  >= b)  bf16, b=1..7 ; SL[:, 0, :] = ones   (7 VE passes)
  Gram: for j in 0..127:  C += SH[:, :, 16j:16j+16].T @ SL[:, :, 16j:16j+16]
     -> C[(a,g), (b,g')] in PSUM [128, 128], accumulated over j
  Cm = C * blockmask (delta_{g,g'})            (VE, 1 op)
  Cred[(a,g), b] = sum_{g'} Cm[., b*16+g']     (VE strided reduce)
  T2[a, b] = E8.T @ Cred  (PE, g-sum)  -> copy into T2all[:, 8s:8s+8]
Epilogue:
  U1 = D @ T2all (PE left-diff), scale 1/N (VE)
  right-diff along b (VE shifted subtract)  -> histn[alpha, (s, beta)]
  h1 = sum_beta W1beta.T @ HH[:, :, beta]  (8 accumulating PE MMs)
  relu+b1 (ACT), W2 MM (PE), +b2 (ACT), DMA out [128, 16].
"""

import numpy as np

import concourse.bacc as bacc
import concourse.mybir as mybir
from concourse.mybir import AluOpType
from concourse.tile import TileContext
from concourse.bass_utils import run_bass_kernel_spmd

HIST_BINS = 64
VMAX = 255.0
SCALE = float(np.float32(HIST_BINS / VMAX))
B, H, W = 128, 512, 512
N_CORES = 8
SPC = B // N_CORES            # 16 samples per core
NPEL = H * W                  # 262144
P = 128
PF = NPEL // P                # 2048 free elems per partition
G = 16                        # f-columns per Gram matmul
NMM = PF // G                 # 128 matmuls per sample
ACT_PLANES = [4, 5, 6, 7]     # hi-step planes built on ScalarE as +-1 signs

F32 = mybir.dt.float32
I16 = mybir.dt.int16
BF16 = mybir.dt.bfloat16


def build_kernel():
    nc = bacc.Bacc("TRN2", target_bir_lowering=False)

    x = nc.dram_tensor("x", [SPC, P, PF], F32, kind="ExternalInput")
    w1r = nc.dram_tensor("w1r", [8, 8, 32], F32, kind="ExternalInput")
    w2t = nc.dram_tensor("w2t", [32, P], F32, kind="ExternalInput")
    b1c = nc.dram_tensor("b1c", [32, 1], F32, kind="ExternalInput")
    b2c = nc.dram_tensor("b2c", [P, 1], F32, kind="ExternalInput")
    maskd = nc.dram_tensor("maskd", [P, P], F32, kind="ExternalInput")
    e8d = nc.dram_tensor("e8d", [P, 8], F32, kind="ExternalInput")
    dtd = nc.dram_tensor("dtd", [8, 8], F32, kind="ExternalInput")
    abias = nc.dram_tensor("abias", [P, len(ACT_PLANES)], F32, kind="ExternalInput")
    y = nc.dram_tensor("y", [P, SPC], F32, kind="ExternalOutput")

    with TileContext(nc) as tc:
        with (
            tc.tile_pool(name="xp", bufs=3) as xp,
            tc.tile_pool(name="idxp", bufs=3) as idxp,
            tc.tile_pool(name="wk", bufs=3) as wk,
            tc.tile_pool(name="sm", bufs=1) as sm,
            tc.tile_pool(name="ps", bufs=2, space="PSUM") as ps,
            tc.tile_pool(name="ps1", bufs=1, space="PSUM") as ps1,
        ):
            w2t_sb = sm.tile([32, P], F32)
            nc.sync.dma_start(out=w2t_sb[:], in_=w2t[:])
            b1_sb = sm.tile([32, 1], F32)
            nc.sync.dma_start(out=b1_sb[:], in_=b1c[:])
            b2_sb = sm.tile([P, 1], F32)
            nc.sync.dma_start(out=b2_sb[:], in_=b2c[:])
            mask_sb = sm.tile([P, P], F32)
            nc.sync.dma_start(out=mask_sb[:], in_=maskd[:])
            e8_sb = sm.tile([P, 8], F32)
            nc.sync.dma_start(out=e8_sb[:], in_=e8d[:])
            dt_sb = sm.tile([8, 8], F32)
            nc.sync.dma_start(out=dt_sb[:], in_=dtd[:])
            w1r_sb = sm.tile([8, 8, 32], F32)
            nc.sync.dma_start(out=w1r_sb[:], in_=w1r[:])
            abias_sb = sm.tile([P, len(ACT_PLANES)], F32)
            nc.sync.dma_start(out=abias_sb[:], in_=abias[:])

            # double-buffered step tensors; ones plane written once each
            sh_tiles = [sm.tile([P, NMM, 8, G], BF16, name=f"sh{i}", tag=f"sh{i}") for i in range(2)]
            sl_tiles = [sm.tile([P, NMM, 8, G], BF16, name=f"sl{i}", tag=f"sl{i}") for i in range(2)]
            for i in range(2):
                nc.vector.memset(sh_tiles[i][:, :, 0, :], 1.0)
                nc.vector.memset(sl_tiles[i][:, :, 0, :], 1.0)

            t2all_sb = sm.tile([8, 8 * SPC], F32)

            for s in range(SPC):
                xt = xp.tile([P, PF], F32)
                nc.sync.dma_start(out=xt[:], in_=x[s])

                idx_t = idxp.tile([P, PF], I16, tag="idx")
                nc.vector.tensor_scalar(
                    idx_t[:], xt[:], SCALE, 0.5, AluOpType.mult, AluOpType.subtract
                )
                lo_t = idxp.tile([P, PF], I16, tag="lo")
                nc.vector.tensor_scalar(
                    lo_t[:], idx_t[:], 7, None, AluOpType.bitwise_and
                )

                SH = sh_tiles[s % 2]
                SL = sl_tiles[s % 2]
                idx_v = idx_t[:].rearrange("p (j g) -> p j g", g=G)
                lo_v = lo_t[:].rearrange("p (j g) -> p j g", g=G)
                for a in range(1, 8):
                    if a in ACT_PLANES:
                        continue
                    nc.vector.tensor_scalar(
                        SH[:, :, a, :], idx_v, float(8 * a), None, AluOpType.is_ge
                    )
                for i, a in enumerate(ACT_PLANES):
                    # sign(idx - 8a + 0.5) = 2*(idx >= 8a) - 1
                    nc.scalar.activation(
                        SH[:, :, a, :],
                        idx_v,
                        mybir.ActivationFunctionType.Sign,
                        bias=abias_sb[:, i : i + 1],
                        scale=1.0,
                    )
                for b in range(1, 8):
                    nc.vector.tensor_scalar(
                        SL[:, :, b, :], lo_v, float(b), None, AluOpType.is_ge
                    )

                c_ps = ps.tile([P, P], F32, tag="cps")
                for j in range(NMM):
                    nc.tensor.matmul(
                        c_ps[:],
                        SH[:, j].rearrange("p a g -> p (a g)"),
                        SL[:, j].rearrange("p a g -> p (a g)"),
                        start=(j == 0),
                        stop=(j == NMM - 1),
                    )

                cm = wk.tile([P, P], F32, tag="cm")
                nc.vector.tensor_tensor(
                    cm[:], c_ps[:], mask_sb[:], AluOpType.mult
                )
                cred = wk.tile([P, 8], F32, tag="cred")
                nc.vector.tensor_reduce(
                    out=cred[:],
                    in_=cm[:].rearrange("p (b g) -> p b g", g=G),
                    op=AluOpType.add,
                    axis=mybir.AxisListType.X,
                )
                t2_ps = ps.tile([8, 8], F32, tag="t2")
                nc.tensor.matmul(
                    t2_ps[:], e8_sb[:], cred[:], start=True, stop=True
                )
                nc.vector.tensor_copy(
                    t2all_sb[:, 8 * s : 8 * (s + 1)], t2_ps[:]
                )

            # left diff: U1 = D @ T2all
            u1_ps = ps1.tile([8, 8 * SPC], F32)
            nc.tensor.matmul(u1_ps[:], dt_sb[:], t2all_sb[:], start=True, stop=True)
            u1_sb = sm.tile([8, SPC, 8], F32)
            nc.vector.tensor_scalar(
                u1_sb[:].rearrange("p s b -> p (s b)"),
                u1_ps[:],
                1.0 / NPEL,
                None,
                AluOpType.mult,
            )
            # right diff along b
            hh = sm.tile([8, SPC, 8], F32)
            nc.vector.tensor_tensor(
                hh[:, :, 0:7], u1_sb[:, :, 0:7], u1_sb[:, :, 1:8],
                AluOpType.subtract,
            )
            nc.vector.tensor_copy(hh[:, :, 7:8], u1_sb[:, :, 7:8])

            # h1 = sum_beta W1beta.T @ HH[:, :, beta]
            h1_ps = ps1.tile([32, SPC], F32)
            for beta in range(8):
                nc.tensor.matmul(
                    h1_ps[:],
                    w1r_sb[:, beta, :],
                    hh[:, :, beta],
                    start=(beta == 0),
                    stop=(beta == 7),
                )
            h1r_sb = sm.tile([32, SPC], F32)
            nc.scalar.activation(
                h1r_sb[:], h1_ps[:], mybir.ActivationFunctionType.Relu,
                bias=b1_sb[:], scale=1.0,
            )
            out_ps = ps1.tile([P, SPC], F32)
            nc.tensor.matmul(out_ps[:], w2t_sb[:], h1r_sb[:], start=True, stop=True)
            out_sb = sm.tile([P, SPC], F32)
            nc.scalar.activation(
                out_sb[:], out_ps[:], mybir.ActivationFunctionType.Identity,
                bias=b2_sb[:], scale=1.0,
            )
            nc.sync.dma_start(out=y[:], in_=out_sb[:])

    nc.compile()
    return nc


_NC_CACHE = {}


def kernel(grad_map, W1, b1, W2, b2, _trace=False):
    grad_map = np.ascontiguousarray(grad_map, dtype=np.float32)
    W1 = np.asarray(W1, dtype=np.float32)
    b1 = np.asarray(b1, dtype=np.float32)
    W2 = np.asarray(W2, dtype=np.float32)
    b2 = np.asarray(b2, dtype=np.float32)

    if "nc" not in _NC_CACHE:
        _NC_CACHE["nc"] = build_kernel()
    nc = _NC_CACHE["nc"]

    w1r = np.ascontiguousarray(W1.T.reshape(8, 8, 32))  # [alpha, beta, j]
    w2t = np.ascontiguousarray(W2.T)
    b1c = np.ascontiguousarray(b1.reshape(32, 1))
    b2c = np.ascontiguousarray(b2.reshape(128, 1))
    maskd = np.ascontiguousarray(
        np.kron(np.ones((8, 8), np.float32), np.eye(G, dtype=np.float32))
    )
    e8d = np.ascontiguousarray(
        np.kron(np.eye(8, dtype=np.float32), np.ones((G, 1), np.float32))
    )
    dmat = np.eye(8, dtype=np.float32) - np.eye(8, k=1, dtype=np.float32)
    # fold the +-1-sign correction for ACT planes into the left-diff:
    # T2_true[a,:] = 0.5*T2_meas[a,:] + 0.5*T2_meas[0,:] for a in ACT_PLANES
    rmat = np.eye(8, dtype=np.float32)
    for a in ACT_PLANES:
        rmat[a, a] = 0.5
        rmat[a, 0] = 0.5
    dtd = np.ascontiguousarray((dmat @ rmat).T)

    abias_h = np.tile(
        np.array([0.5 - 8.0 * a for a in ACT_PLANES], np.float32)[None, :],
        (P, 1),
    )
    xs = grad_map.reshape(N_CORES, SPC, P, PF)
    in_maps = [
        {"x": np.ascontiguousarray(xs[c]), "w1r": w1r, "w2t": w2t,
         "b1c": b1c, "b2c": b2c, "maskd": maskd, "e8d": e8d, "dtd": dtd,
         "abias": abias_h}
        for c in range(N_CORES)
    ]

    res = run_bass_kernel_spmd(
        nc, in_maps, core_ids=list(range(N_CORES)), trace=_trace
    )
    out = np.concatenate([r["y"].T for r in res.results], axis=0)
    if _trace:
        return out, res
    return out
